# revision 16
# baseline (speedup 1.0000x reference)
"""Trainium2 Bass kernel for nn_DepthLoss (focal loss over box-union mask).

Math (per element, t = mask in {0,1}, p = depth in [0,1)):
  y = (2t-1)(2p-1) = 1 - 2x  where x = |t - p|
  loss_e = sigmoid(y)^2 * softplus(y)  ~=  r0 + r3 * Q(x)
  Q(x) = x*(rho1 + x*(rho2 + x))   (monic cubic; LS fit on x in [0,1])
  loss = mean(loss_e) = r3 * sum(Q) / M + r0     (r0, r3 applied on host)

Device pipeline per core (b-split 2 x h-split 4, 12 tiles of [128,2048] f32):
  PE  : counts = rowInd^T @ colInd   (bf16 indicator matmul into PSUM)
  DVE : ONE fused custom op per tile:
          m = (counts > 0); x = |p - m|; accum += x*(C0 + x*(C1 + x))
  Indicators come from an Idx-based custom op (no iota, no gpsimd):
          out = (Idx >= Src0) & (Idx < C0)   with Src0 = bound broadcast.
Host: bbox bounds preprocessing, final r3*sum/M + r0.
Approx: cubic fit max abs err 3.1e-3 pointwise; mean error ~2e-7 (x ~ U(0,1)).
"""

import numpy as np

B, C, H, W = 8, 1, 1536, 2048
NUM_GTS = 64
LOSS_WEIGHT = 1.0
NCORES = 8
HSPLIT = 4          # h blocks of 384 rows
BSPLIT = 2          # groups of 4 images
ROWS = H // HSPLIT  # 384
CBLK = ROWS // 128  # 3 row-blocks of 128 per h block
NB = B // BSPLIT    # 4 images per core
NTILES = NB * CBLK  # 12 tiles of [128, 2048] per core

# LS cubic fit of f(x) = sig(1-2x)^2 * softplus(1-2x) on x in [0,1]:
#   f ~= R0 + R3 * (x*(RHO1 + x*(RHO2 + x)))
R0 = 0.7049675582884715
R3 = -0.32424734876802186
RHO1 = 4.948403160619334
RHO2 = -3.8489944218920242

_COMPILED = {}


def _register_dve_ops():
    """Register the two custom DVE ops (idempotent)."""
    from operator import add as _add

    from concourse import dve_ops
    from concourse.dve_spec import (
        AluOp, Bin, C0, C1, Idx, Spec, Src0, Src1, Zero, lower, _has_src1,
    )
    from concourse.dve_uop import DveOpSpec

    def _indx_ref(in0, in1, s0, s1, imm2):
        n = in0.shape[-1]
        idx = np.arange(n, dtype=np.float32)[None, :]
        return ((idx >= in0) & (idx < s0)).astype(np.float32)

    def _foc_ref(in0, in1, s0, s1, imm2):
        p = in0.astype(np.float32)
        m = (in1 > 0).astype(np.float32)
        x = np.abs(p - m)
        q = (x * (s0 + x * (s1 + x))).astype(np.float32)
        return q, q.reshape(q.shape[0], -1).sum(axis=-1, keepdims=True)

    # indicator: (Idx >= Src0) & (Idx < C0); Src0 = lower bound broadcast
    ind_body = (Idx >= Src0) & (Idx < C0)
    # fused focal: m = cnt>0; x = |p - m|; Q = x*(C0 + x*(C1 + x)); accum add
    x = Bin(AluOp.ABSOLUTE_DIFF, Src0, Zero < Src1)
    foc_body = ((x + C1) * x + C0) * x

    specs = {
        "ANT_DL_INDX": Spec(body=ind_body, reference=_indx_ref),
        "ANT_DL_FOC": Spec(body=foc_body, accum=_add, reference=_foc_ref),
    }

    out = {}
    existing = {op.name: op for op in dve_ops.OPS}
    for name, spec in specs.items():
        if name in existing:
            out[name] = existing[name]
            continue
        shas = {}
        for ver in ("v3", "v4"):
            try:
                s = DveOpSpec(name=name, opcode=1, uops=lower(spec, ver=ver),
                              rd1_en=_has_src1(spec))
                shas[ver] = s.sha(ver)
            except Exception:
                pass
        op = dve_ops.DveOp(name, spec, False, uops_sha=shas)
        dve_ops.OPS.append(op)
        dve_ops.CUSTOM_DVE_SPECS[name] = spec
        dve_ops._SUB_OPCODE_FOR_NAME[name] = dve_ops._CUSTOM_DVE_ROW_BASE + len(dve_ops.OPS) - 1
        out[name] = op
    return out


def _build_program():
    """Build + compile the per-core Bass program. Same program for all 8 cores."""
    from contextlib import ExitStack

    import concourse.bass as bass
    import concourse.mybir as mybir
    import concourse.tile as tile
    from concourse import bacc
    from concourse.bass import broadcast_tensor_aps

    ops = _register_dve_ops()
    INDX, FOC = ops["ANT_DL_INDX"], ops["ANT_DL_FOC"]

    f32, bf16 = mybir.dt.float32, mybir.dt.bfloat16

    nc = bacc.Bacc("TRN2", target_bir_lowering=False, debug=False,
                   num_devices=NCORES)

    depth_d = nc.dram_tensor("depth_in", [NB * ROWS, W], f32, kind="ExternalInput").ap()
    # bnds: [64, 10] f32 host-precomputed:
    #   [0] rowLo = tl_y-1-hoff      [1] rowHi = max(br_y,1)-hoff
    #   [2+2w] colLo-512w  [3+2w] colHi-512w  for w in 0..3
    # (col bounds pre-shifted per 512-chunk because Idx restarts per call)
    bnds_d = nc.dram_tensor("bnds_in", [NUM_GTS, 10], f32, kind="ExternalInput").ap()
    acc_d = nc.dram_tensor("acc_out", [128, NTILES], f32, kind="ExternalOutput").ap()

    with tile.TileContext(nc) as tc, ExitStack() as ctx:
        const = ctx.enter_context(tc.tile_pool(name="const", bufs=1))
        ppool = ctx.enter_context(tc.tile_pool(name="p", bufs=8))
        psum = ctx.enter_context(
            tc.tile_pool(name="cnt", bufs=2, space=bass.MemorySpace.PSUM))

        # bnds leads the Sync HWDGE queue (1KB; delays sync tiles by ~0.1us)
        bnds = const.tile([NUM_GTS, 10], f32)
        nc.sync.dma_start(bnds[:], bnds_d[:])

        # ---- indicators via Idx custom op (values {0,1} in bf16 for PE) ----
        # row first (feeds LDWEIGHTS), col in 512-wide chunks (bounds
        # pre-shifted per chunk host-side) so each matmul starts as soon as
        # its chunk is written
        row1 = const.tile([NUM_GTS, ROWS], bf16)
        in0r, _ = broadcast_tensor_aps(bnds[:, 0:1], row1[:])
        nc.vector._custom_dve(INDX, out=row1[:], in0=in0r, s0=bnds[:, 1:2])
        col1 = const.tile([NUM_GTS, W], bf16)
        for wc in range(W // 512):
            cs = slice(512 * wc, 512 * (wc + 1))
            in0c, _ = broadcast_tensor_aps(bnds[:, 2 + 2 * wc:3 + 2 * wc],
                                           col1[:, cs])
            nc.vector._custom_dve(INDX, out=col1[:, cs], in0=in0c,
                                  s0=bnds[:, 3 + 2 * wc:4 + 2 * wc])

        acc = const.tile([128, NTILES], f32)

        # Two HWDGE queues stream in parallel: Scalar (images 2-3) leads the
        # DMA-engine arbitration, Sync (images 0-1 + bnds) lags — so consume
        # scalar's tiles first in each group and END on sync's (laggard-last
        # keeps the final FOC adjacent to the final DMA byte).
        for g in range(CBLK):
            for b in (2, 0, 3, 1):
                ti = CBLK * b + g
                eng = nc.scalar if b >= 2 else nc.sync
                p = ppool.tile([128, W], f32)
                eng.dma_start(p[:], depth_d[128 * ti:128 * (ti + 1), :])
                if b == 2:
                    cnt = psum.tile([128, W], f32)  # 4 PSUM banks
                    for wc in range(W // 512):
                        cs = slice(512 * wc, 512 * (wc + 1))
                        nc.tensor.matmul(cnt[:, cs],
                                         row1[:, 128 * g:128 * (g + 1)],
                                         col1[:, cs], start=True, stop=True)
                nc.vector._custom_dve(FOC, out=p[:], in0=p[:],
                                      in1=cnt[:], s0=RHO1, s1=RHO2,
                                      accum_out=acc[:, ti:ti + 1])

        nc.sync.dma_start(acc_d[:], acc[:])

    nc.compile()
    return nc


def _get_compiled():
    if "nc" not in _COMPILED:
        _COMPILED["nc"] = _build_program()
    return _COMPILED["nc"]


def _in_maps(depth, bbox):
    tx = bbox[:, 0].astype(np.float32)
    ty = bbox[:, 1].astype(np.float32)
    bx = bbox[:, 2].astype(np.float32)
    by = bbox[:, 3].astype(np.float32)
    colLo = tx - 1.0
    colHi = np.maximum(bx, float(B))
    maps = []
    for k in range(NCORES):
        bg, hb = k // HSPLIT, k % HSPLIT
        hoff = float(ROWS * hb)
        shard = np.ascontiguousarray(
            depth[NB * bg:NB * (bg + 1), 0, ROWS * hb:ROWS * (hb + 1), :]
            .reshape(NB * ROWS, W))
        cols = [ty - 1.0 - hoff, np.maximum(by, float(C)) - hoff]
        for wc in range(4):
            cols += [colLo - 512.0 * wc, colHi - 512.0 * wc]
        bnds = np.stack(cols, axis=1).astype(np.float32)
        maps.append({"depth_in": shard, "bnds_in": np.ascontiguousarray(bnds)})
    return maps


def run_on_device(depth, bbox_list, trace=False, **trace_kwargs):
    """Run the SPMD kernel on 8 cores; returns (loss_scalar, BassKernelResults)."""
    from concourse import bass_utils

    depth = np.asarray(depth, dtype=np.float32)
    bbox = np.asarray(bbox_list, dtype=np.float64)
    nc = _get_compiled()
    res = bass_utils.run_bass_kernel_spmd(
        nc, _in_maps(depth, bbox), core_ids=list(range(NCORES)),
        trace=trace, **trace_kwargs)
    total = sum(float(r["acc_out"].astype(np.float64).sum()) for r in res.results)
    loss = (R3 * total / float(B * C * H * W) + R0) * LOSS_WEIGHT
    return np.asarray(loss, dtype=np.float32), res


def kernel(depth, bbox_list, device=None, **_):
    loss, _res = run_on_device(depth, bbox_list, trace=False)
    return loss
